# revision 47
# baseline (speedup 1.0000x reference)
"""Trainium2 Bass kernel for nn_CAE_21242908246023 (moe_routing).

Computation (B=16384, D=5000, L=64):
  h_base = expr @ W_base.T                     [B, L]
  logits = h_base @ W_base                     [B, D]
  for ctx in (batch[card 24], cell[card 10]):
      shared = expr @ W_enc.T                  [B, L]
      h_f    = einsum('bl,bml->bm', shared, W_heads[src])
      dec    = einsum('bl,bml->bm', h_f,    W_heads[tgt])
      logits += (dec @ W_dec.T) * 0.0159

Strategy: data-parallel over B across 8 cores (2048 rows each), weights
replicated (per the sharding hint; no collectives needed).  bf16 matmuls
with fp32 PSUM accumulation; bf16 output upcast to fp32 on host.
Per core: expr is pre-transposed/padded on host to [5120, 2048] so the
encoder's contraction-major tiles load as plain contiguous DMAs; one
fused encoder pass produces all three 64-dim latents in transposed
layout ([latent, row], which is exactly the lhsT the decoder needs);
per-row expert routing is computed all-experts on the tensor engine and
selected with host-built one-hot masks (broadcast mask multiply + tree
reduction on DVE), software-pipelined at depth 2 across row tiles so the
PE never stalls on a single tile's select chain; the three decoder
matmuls are fused into one PSUM accumulation against a host-stacked
[192, 5000] weight; output tiles stream out as contiguous 1.25MB DMAs.
Deep input prefetch (24 tile buffers) keeps the next quarter's loads
streaming during the current quarter's routing phase.
"""

from contextlib import ExitStack

import ml_dtypes
import numpy as np

import concourse.bacc as bacc
import concourse.bass as bass
import concourse.mybir as mybir
import concourse.tile as tile
from concourse._compat import with_exitstack
from concourse.bass_utils import run_bass_kernel_spmd

BF16 = ml_dtypes.bfloat16

B, D, L = 16384, 5000, 64
CARD_B, CARD_C = 24, 10
DEC_SCALE = 0.0159
N_CORES = 8
R = B // N_CORES          # rows per core
DP = 5120                 # D padded to a multiple of 128
NK = DP // 128            # contraction chunks (40)
QW = 512                  # encoder quarter width (rows)
NOUT = 10                 # decoder output chunks of 500
OW = D // NOUT            # 500

f32 = mybir.dt.float32
bf16 = mybir.dt.bfloat16


def _ap3(ap, outer, inner):
    """[P, outer*inner] AP -> [P, outer, inner] view."""
    pdim, fdim = ap.ap[0], ap.ap[1]
    assert fdim[1] == outer * inner and fdim[0] == 1
    return bass.AP(ap.tensor, ap.offset, [pdim, [inner, outer], [1, inner]])


def _bcast(ap, n):
    """[P, E] AP -> [P, E, n] broadcast view (step-0 inner dim)."""
    return bass.AP(ap.tensor, ap.offset, ap.ap + [[0, n]])


@with_exitstack
def _kernel(ctx, tc, rows, io, ab="full", xbufs=24, lp_dve=0,
            enc_bufs=1, lp_bufs=2, tp_share=False, hp_bufs=2,
            tmp_bufs=2, small_bufs=3, osb_bufs=2, adds_gpsimd=False,
            wenc_split=4, fused_sel=False, qw=QW, dma_red=False,
            rowpack=False):
    nc = tc.nc
    nq = rows // qw
    nt = rows // 128

    (x, wenc, wout1, wout2, wstb, wstc, msbs, msbt, mscs, msct, ident,
     ident32, y) = io

    consts = ctx.enter_context(tc.tile_pool(name="consts", bufs=1))

    def load_const(ap, dtype):
        t = consts.tile(list(ap.shape), dtype, tag=ap.tensor.name)
        nc.sync.dma_start(t[:], ap)
        return t

    # split the 2MB encoder-weight load so the first matmuls don't wait
    # for the whole tensor
    assert NK % wenc_split == 0
    ws = NK // wenc_split
    wenc_ts = []
    for i in range(wenc_split):
        t = consts.tile([128, ws * 192], bf16, tag=f"wenc{i}")
        nc.sync.dma_start(t[:], wenc[:, i * ws * 192:(i + 1) * ws * 192])
        wenc_ts.append(t)
    wout1_t = load_const(wout1, bf16)
    wout2_t = load_const(wout2, bf16)
    wstb_t = load_const(wstb, bf16)
    wstc_t = load_const(wstc, bf16)
    msbs_t = load_const(msbs, f32)
    msbt_t = load_const(msbt, f32)
    mscs_t = load_const(mscs, f32)
    msct_t = load_const(msct, f32)
    ident_t = load_const(ident, bf16)
    ident32_t = load_const(ident32, f32)

    lat = ctx.enter_context(tc.tile_pool(name="lat", bufs=1))
    shp = 128 if rowpack else 64
    shB = lat.tile([shp, rows], bf16, tag="shB")   # shared_batch^T (x2 if rowpack)
    shC = lat.tile([shp, rows], bf16, tag="shC")   # shared_cell^T
    zT1 = lat.tile([128, rows], bf16, tag="zT1")   # [h_base^T ; dec_b^T]
    zT2 = lat.tile([64, rows], bf16, tag="zT2")    # dec_c^T

    xpool = ctx.enter_context(tc.tile_pool(name="xT", bufs=xbufs))
    encps = ctx.enter_context(tc.tile_pool(name="encps", bufs=enc_bufs, space="PSUM"))
    headps = ctx.enter_context(tc.tile_pool(name="headps", bufs=hp_bufs, space="PSUM"))
    tpps = headps if tp_share else ctx.enter_context(
        tc.tile_pool(name="tpps", bufs=2, space="PSUM"))
    outps = ctx.enter_context(tc.tile_pool(name="outps", bufs=lp_bufs, space="PSUM"))
    tmpp = ctx.enter_context(tc.tile_pool(name="tmpp", bufs=tmp_bufs))
    small = ctx.enter_context(tc.tile_pool(name="small", bufs=small_bufs))
    opool = ctx.enter_context(tc.tile_pool(name="osb", bufs=osb_bufs))

    def select(ps_tiles, mask_t, moff, card, tag, out_ap):
        """Per-row expert selection: out[p, m] = sum_e mask[p, e] * ps[p, e*64+m].

        ps_tiles: list of ([128, w] psum AP, expert offset, n_experts).
        Writes the [128, 64] result to out_ap (bf16).
        """
        if ab == "nosel":
            nc.vector.tensor_copy(out_ap, ps_tiles[0][0][:, 0:64])
            return
        if fused_sel:
            # masked products land in [m, e] layout; one innermost-axis
            # reduce replaces the whole expert add-tree (out must be f32)
            tmp = tmpp.tile([128, 64 * card], bf16, tag=f"tmp{tag}")

            def _mk(base, dims):
                return bass.AP(base.tensor, base.offset, [base.ap[0]] + dims)

            for ps, e0, ne in ps_tiles:
                nc.vector.tensor_mul(
                    _mk(tmp[:, e0:], [[card, 64], [1, ne]]),
                    _mk(ps, [[1, 64], [64, ne]]),
                    _mk(mask_t[:, moff + e0:], [[0, 64], [1, ne]]),
                )
            nc.vector.reduce_sum(
                out_ap, _mk(tmp[:], [[card, 64], [1, card]]),
                axis=mybir.AxisListType.X)
            return
        tmp = tmpp.tile([128, card * 64], bf16, tag=f"tmp{tag}")
        if ab == "natsel":
            for ps, e0, ne in ps_tiles:
                for j in range(ne):
                    e = e0 + j
                    nc.vector.tensor_scalar_mul(
                        tmp[:, e * 64:(e + 1) * 64],
                        ps[:, j * 64:(j + 1) * 64],
                        mask_t[:, moff + e:moff + e + 1])
        else:
            for ps, e0, ne in ps_tiles:
                nc.vector.tensor_mul(
                    _ap3(tmp[:, e0 * 64:(e0 + ne) * 64], ne, 64),
                    _ap3(ps, ne, 64),
                    _bcast(mask_t[:, moff + e0:moff + e0 + ne], 64),
                )
        if dma_red:
            # expert reduction on the DMA engines: one SWDGE transfer with a
            # step-0 destination accumulates all expert blocks (per-partition
            # descriptor order makes the read-modify-write race-free)
            nc.gpsimd.memset(out_ap, 0.0)
            in3 = _ap3(tmp[:], card, 64)
            out3 = bass.AP(out_ap.tensor, out_ap.offset,
                           [out_ap.ap[0], [0, card], [1, 64]])
            nc.gpsimd.dma_start(out3, in3, accum_op=mybir.AluOpType.add)
            return
        # tree-reduce over experts
        eng = nc.gpsimd if adds_gpsimd else nc.vector

        def halve(src, n):
            h = n // 2
            dst = small.tile([128, h * 64], bf16, tag=f"acc{tag}{h}")
            eng.tensor_add(dst[:], src[:, :h * 64], src[:, h * 64:2 * h * 64])
            return dst, src[:, 2 * h * 64:] if n % 2 else None

        cur, n = tmp, card
        extras = []
        while n > 1:
            cur, rem = halve(cur, n)
            if rem is not None:
                extras.append(rem)
            n //= 2
        cur = cur[:]
        if extras:
            for ex in extras[:-1]:
                nxt = small.tile([128, 64], bf16, tag=f"hx{tag}")
                eng.tensor_add(nxt[:], cur, ex)
                cur = nxt[:]
            eng.tensor_add(out_ap, cur, extras[-1])
        else:
            eng.tensor_copy(out_ap, cur)

    def head_chunks(src2, b, wst_t, card):
        """All-experts matmuls; with rowpack, consecutive chunks go to
        alternating PE row-groups (partitions 0:64 / 64:128) and run
        concurrently."""
        res = []
        total = card * 64
        c0 = 0
        ci = 0
        while c0 < total:
            w = min(512, total - c0)
            ps = headps.tile([128, 512], f32, tag="hps")
            half = (ci % 2) * 64 if rowpack else 0
            lhsT = (src2[half:half + 64, b:b + 128] if b is not None
                    else src2[half:half + 64, :])
            nc.tensor.matmul(ps[:, :w], lhsT,
                             wst_t[half:half + 64, c0:c0 + w],
                             start=True, stop=True)
            res.append((ps[:, :w], c0 // 64, w // 64))
            c0 += w
            ci += 1
        return res

    def transpose_pair(src_t, tag):
        """[128, 128] sbuf pair -> [128, 128] psum via one PE transpose."""
        dt = f32 if fused_sel else bf16
        tp = tpps.tile([128, 128], dt, tag="hps" if tp_share else "tp")
        nc.tensor.transpose(tp[:], src_t[:],
                            ident32_t[:] if fused_sel else ident_t[:])
        return tp

    cp = mybir.ActivationFunctionType.Copy
    state = {}

    def encode_quarter(q):
        b0 = q * qw
        h1 = encps.tile([128, qw], f32, tag="h1")
        h2 = encps.tile([64, qw], f32, tag="h2")
        for k in range(NK):
            xt = xpool.tile([128, qw], bf16, tag="xt")
            nc.sync.dma_start(xt[:], x[k * 128:(k + 1) * 128, b0:b0 + qw])
            wt = wenc_ts[k // ws]
            ko = (k % ws) * 192
            for c0 in range(0, qw, 512):
                cw = min(512, qw - c0)
                nc.tensor.matmul(h1[:, c0:c0 + cw], wt[:, ko:ko + 128],
                                 xt[:, c0:c0 + cw],
                                 start=(k == 0), stop=(k == NK - 1))
                nc.tensor.matmul(h2[:, c0:c0 + cw], wt[:, ko + 128:ko + 192],
                                 xt[:, c0:c0 + cw],
                                 start=(k == 0), stop=(k == NK - 1))
        nc.scalar.activation(zT1[0:64, b0:b0 + qw], h1[0:64, :], cp)
        nc.scalar.activation(shB[0:64, b0:b0 + qw], h1[64:128, :], cp)
        nc.scalar.activation(shC[0:64, b0:b0 + qw], h2[0:64, :], cp)
        if rowpack:
            nc.scalar.activation(shB[64:128, b0:b0 + qw], h1[64:128, :], cp)
            nc.scalar.activation(shC[64:128, b0:b0 + qw], h2[0:64, :], cp)

    def phase1(t):
        """src heads: all-experts mm + select -> hfp [128, 128]."""
        b = t * 128
        hfp = small.tile([128, 128], f32 if fused_sel else bf16, tag="hfp")
        ps1 = head_chunks(shB, b, wstb_t, CARD_B)
        select(ps1, msbs_t, t * CARD_B, CARD_B, "b1", hfp[:, 0:64])
        ps1 = head_chunks(shC, b, wstc_t, CARD_C)
        select(ps1, mscs_t, t * CARD_C, CARD_C, "c1", hfp[:, 64:128])
        state[t] = hfp

    def phase2(t):
        """transpose hfp, tgt heads: mm + select -> dcp."""
        hfp = state.pop(t)
        hfT = transpose_pair(hfp, "s1")
        hp = 128 if rowpack else 64
        hfTb = small.tile([hp, 128], bf16, tag="hfTb")
        hfTc = small.tile([hp, 128], bf16, tag="hfTc")
        nc.scalar.activation(hfTb[0:64, :], hfT[0:64, :], cp)
        nc.scalar.activation(hfTc[0:64, :], hfT[64:128, :], cp)
        if rowpack:
            nc.scalar.activation(hfTb[64:128, :], hfT[0:64, :], cp)
            nc.scalar.activation(hfTc[64:128, :], hfT[64:128, :], cp)
        dcp = small.tile([128, 128], f32 if fused_sel else bf16, tag="dcp")
        ps2 = head_chunks(hfTb, None, wstb_t, CARD_B)
        select(ps2, msbt_t, t * CARD_B, CARD_B, "b2", dcp[:, 0:64])
        ps2 = head_chunks(hfTc, None, wstc_t, CARD_C)
        select(ps2, msct_t, t * CARD_C, CARD_C, "c2", dcp[:, 64:128])
        state[t] = dcp

    def phase3(t):
        """transpose dcp into zT, decoder matmuls, drain, store."""
        b = t * 128
        if ab == "nohead":
            nc.vector.tensor_copy(zT1[64:128, b:b + 128], shB[:, b:b + 128])
            nc.vector.tensor_copy(zT2[:, b:b + 128], shC[:, b:b + 128])
        else:
            dcp = state.pop(t)
            dcT = transpose_pair(dcp, "s2")
            nc.scalar.activation(zT1[64:128, b:b + 128], dcT[0:64, :], cp)
            nc.scalar.activation(zT2[:, b:b + 128], dcT[64:128, :], cp)
        osb = opool.tile([128, D], bf16, tag="osb")
        for n in range(NOUT):
            lp = outps.tile([128, OW], f32, tag="lp")
            nc.tensor.matmul(lp[:], zT1[:, b:b + 128],
                             wout1_t[:, n * OW:(n + 1) * OW],
                             start=True, stop=False)
            nc.tensor.matmul(lp[:], zT2[:, b:b + 128],
                             wout2_t[:, n * OW:(n + 1) * OW],
                             start=False, stop=True)
            if n % NOUT < lp_dve:
                nc.vector.tensor_copy(osb[:, n * OW:(n + 1) * OW], lp[:])
            else:
                nc.scalar.activation(osb[:, n * OW:(n + 1) * OW], lp[:], cp)
        nc.scalar.dma_start(y[b:b + 128, :], osb[:])

    def tiles_of(q):
        return range(q * (qw // 128), (q + 1) * (qw // 128))

    for q in range(nq):
        encode_quarter(q)
        # software pipeline (depth 2) so PE never stalls on a single
        # tile's select chain: stage-1 of tile t issues ahead of
        # stage-2 of t-1 and the decoder of t-2.
        if ab == "nohead":
            for t in tiles_of(q):
                phase3(t)
        else:
            for t in tiles_of(q):
                phase1(t)
                if t - 1 >= 0:
                    phase2(t - 1)
                if t - 2 >= 0:
                    phase3(t - 2)
    if ab != "nohead":
        phase2(nt - 1)
        phase3(nt - 2)
        phase3(nt - 1)


def _declare(nc, rows):
    def di(name, shape, dt):
        return nc.dram_tensor(name, shape, dt, kind="ExternalInput").ap()

    x = di("x", [DP, rows], bf16)   # expr^T (host pre-transposed)
    wenc = di("wenc", [128, NK * 192], bf16)
    wout1 = di("wout1", [128, D], bf16)
    wout2 = di("wout2", [64, D], bf16)
    wstb = di("wstb", [128, CARD_B * 64], bf16)
    wstc = di("wstc", [128, CARD_C * 64], bf16)
    nt = rows // 128
    msbs = di("msbs", [128, nt * CARD_B], f32)
    msbt = di("msbt", [128, nt * CARD_B], f32)
    mscs = di("mscs", [128, nt * CARD_C], f32)
    msct = di("msct", [128, nt * CARD_C], f32)
    ident = di("ident", [128, 128], bf16)
    ident32 = di("ident32", [128, 128], f32)
    y = nc.dram_tensor("y", [rows, D], bf16, kind="ExternalOutput").ap()
    return [x, wenc, wout1, wout2, wstb, wstc, msbs, msbt, mscs, msct,
            ident, ident32, y]


_PROGRAMS = {}


def build_program(rows=R, ab="full", **kw):
    key = (rows, ab, tuple(sorted(kw.items())))
    if key in _PROGRAMS:
        return _PROGRAMS[key]
    nc = bacc.Bacc("TRN2", target_bir_lowering=False, debug=False,
                   num_devices=N_CORES if rows == R else 1)
    io = _declare(nc, rows)
    with tile.TileContext(nc) as tc:
        _kernel(tc, rows, io, ab=ab, **kw)
    nc.compile()
    _PROGRAMS[key] = nc
    return nc


def prep_weights(W_base, W_enc_batch, W_dec_batch, W_heads_batch,
                 W_enc_cell, W_dec_cell, W_heads_cell):
    stackT = np.zeros((DP, 192), np.float32)
    stackT[:D, 0:64] = W_base.T
    stackT[:D, 64:128] = W_enc_batch.T
    stackT[:D, 128:192] = W_enc_cell.T
    wenc = np.ascontiguousarray(
        stackT.reshape(NK, 128, 192).transpose(1, 0, 2).reshape(128, NK * 192)
    ).astype(BF16)
    wout = np.concatenate(
        [W_base, DEC_SCALE * W_dec_batch.T, DEC_SCALE * W_dec_cell.T], axis=0
    ).astype(BF16)
    wstb = np.ascontiguousarray(
        W_heads_batch.transpose(2, 0, 1).reshape(64, CARD_B * 64)).astype(BF16)
    wstb = np.vstack([wstb, wstb])
    wstc = np.ascontiguousarray(
        W_heads_cell.transpose(2, 0, 1).reshape(64, CARD_C * 64)).astype(BF16)
    wstc = np.vstack([wstc, wstc])
    ident = np.eye(128, dtype=BF16)
    return {
        "wenc": wenc, "wout1": np.ascontiguousarray(wout[0:128]),
        "wout2": np.ascontiguousarray(wout[128:192]),
        "wstb": wstb, "wstc": wstc, "ident": ident,
        "ident32": np.eye(128, dtype=np.float32),
    }


def prep_mask(idx, card):
    """[rows] int -> [128, (rows/128)*card] f32 one-hot in SBUF layout."""
    nt = idx.shape[0] // 128
    oh = (idx.reshape(nt, 128)[:, :, None] == np.arange(card)).astype(np.float32)
    return np.ascontiguousarray(oh.transpose(1, 0, 2).reshape(128, nt * card))


def prep_x(expr_rows):
    """[rows, D] f32 -> padded transposed [DP, rows] bf16."""
    xp = np.zeros((DP, expr_rows.shape[0]), BF16)
    xp[:D, :] = expr_rows.astype(BF16).T
    return xp


def kernel(expr, src_batch, tgt_batch, src_cell, tgt_cell,
           W_base, W_enc_batch, W_dec_batch, W_heads_batch,
           W_enc_cell, W_dec_cell, W_heads_cell):
    import os
    # The NTFF trace path needs antenv.axon_hooks, absent in this
    # container; a stray BASS_TRACE=1 would crash the run otherwise.
    os.environ.setdefault("BASS_NEVER_TRACE", "1")
    expr = np.asarray(expr)
    src_batch, tgt_batch = np.asarray(src_batch), np.asarray(tgt_batch)
    src_cell, tgt_cell = np.asarray(src_cell), np.asarray(tgt_cell)
    nc = build_program(R)
    wmap = prep_weights(W_base, W_enc_batch, W_dec_batch, W_heads_batch,
                        W_enc_cell, W_dec_cell, W_heads_cell)
    in_maps = []
    for c in range(N_CORES):
        sl = slice(c * R, (c + 1) * R)
        in_maps.append({
            "x": prep_x(expr[sl]),
            "msbs": prep_mask(src_batch[sl], CARD_B),
            "msbt": prep_mask(tgt_batch[sl], CARD_B),
            "mscs": prep_mask(src_cell[sl], CARD_C),
            "msct": prep_mask(tgt_cell[sl], CARD_C),
            **wmap,
        })
    res = run_bass_kernel_spmd(nc, in_maps, core_ids=list(range(N_CORES)))
    global LAST_RESULT
    LAST_RESULT = res
    out = np.concatenate([res.results[c]["y"] for c in range(N_CORES)], axis=0)
    return np.asarray(out, dtype=np.float32)


LAST_RESULT = None


# revision 48
# speedup vs baseline: 2.0843x; 2.0843x over previous
"""Trainium2 Bass kernel for nn_CAE_21242908246023 (moe_routing).

Computation (B=16384, D=5000, L=64):
  h_base = expr @ W_base.T                     [B, L]
  logits = h_base @ W_base                     [B, D]
  for ctx in (batch[card 24], cell[card 10]):
      shared = expr @ W_enc.T                  [B, L]
      h_f    = einsum('bl,bml->bm', shared, W_heads[src])
      dec    = einsum('bl,bml->bm', h_f,    W_heads[tgt])
      logits += (dec @ W_dec.T) * 0.0159

Strategy: data-parallel over B across 8 cores (2048 rows each), weights
replicated (per the sharding hint; no collectives needed).  bf16 matmuls
with fp32 PSUM accumulation; bf16 output upcast to fp32 on host.
Per core: expr is pre-transposed/padded on host to [5120, 2048] so the
encoder's contraction-major tiles load as plain contiguous DMAs; one
fused encoder pass produces all three 64-dim latents in transposed
layout ([latent, row], which is exactly the lhsT the decoder needs);
per-row expert routing is computed all-experts on the tensor engine and
selected with host-built one-hot masks (broadcast mask multiply + tree
reduction on DVE), software-pipelined at depth 2 across row tiles so the
PE never stalls on a single tile's select chain; the three decoder
matmuls are fused into one PSUM accumulation against a host-stacked
[192, 5000] weight; output tiles stream out as contiguous 1.25MB DMAs.
Deep input prefetch (24 tile buffers) keeps the next quarter's loads
streaming during the current quarter's routing phase.
"""

from contextlib import ExitStack

import ml_dtypes
import numpy as np

import concourse.bacc as bacc
import concourse.bass as bass
import concourse.mybir as mybir
import concourse.tile as tile
from concourse._compat import with_exitstack
from concourse.bass_utils import run_bass_kernel_spmd

BF16 = ml_dtypes.bfloat16

B, D, L = 16384, 5000, 64
CARD_B, CARD_C = 24, 10
DEC_SCALE = 0.0159
N_CORES = 8
R = B // N_CORES          # rows per core
DP = 5120                 # D padded to a multiple of 128
NK = DP // 128            # contraction chunks (40)
QW = 512                  # encoder quarter width (rows)
NOUT = 10                 # decoder output chunks of 500
OW = D // NOUT            # 500

f32 = mybir.dt.float32
bf16 = mybir.dt.bfloat16


def _ap3(ap, outer, inner):
    """[P, outer*inner] AP -> [P, outer, inner] view."""
    pdim, fdim = ap.ap[0], ap.ap[1]
    assert fdim[1] == outer * inner and fdim[0] == 1
    return bass.AP(ap.tensor, ap.offset, [pdim, [inner, outer], [1, inner]])


def _bcast(ap, n):
    """[P, E] AP -> [P, E, n] broadcast view (step-0 inner dim)."""
    return bass.AP(ap.tensor, ap.offset, ap.ap + [[0, n]])


@with_exitstack
def _kernel(ctx, tc, rows, io, ab="full", xbufs=24, lp_dve=0,
            enc_bufs=1, lp_bufs=2, tp_share=False, hp_bufs=2,
            tmp_bufs=2, small_bufs=3, osb_bufs=2, adds_gpsimd=False,
            wenc_split=4, fused_sel=False, qw=QW, dma_red=False,
            rowpack=False, lag1=1, lag2=2):
    nc = tc.nc
    nq = rows // qw
    nt = rows // 128

    (x, wenc, wout1, wout2, wstb, wstc, msbs, msbt, mscs, msct, ident,
     ident32, y) = io

    consts = ctx.enter_context(tc.tile_pool(name="consts", bufs=1))

    def load_const(ap, dtype):
        t = consts.tile(list(ap.shape), dtype, tag=ap.tensor.name)
        nc.sync.dma_start(t[:], ap)
        return t

    # split the 2MB encoder-weight load so the first matmuls don't wait
    # for the whole tensor
    assert NK % wenc_split == 0
    ws = NK // wenc_split
    wenc_ts = []
    for i in range(wenc_split):
        t = consts.tile([128, ws * 192], bf16, tag=f"wenc{i}")
        nc.sync.dma_start(t[:], wenc[:, i * ws * 192:(i + 1) * ws * 192])
        wenc_ts.append(t)
    wout1_t = load_const(wout1, bf16)
    wout2_t = load_const(wout2, bf16)
    wstb_t = load_const(wstb, bf16)
    wstc_t = load_const(wstc, bf16)
    msbs_t = load_const(msbs, f32)
    msbt_t = load_const(msbt, f32)
    mscs_t = load_const(mscs, f32)
    msct_t = load_const(msct, f32)
    ident_t = load_const(ident, bf16)
    ident32_t = load_const(ident32, f32)

    lat = ctx.enter_context(tc.tile_pool(name="lat", bufs=1))
    shp = 128 if rowpack else 64
    shB = lat.tile([shp, rows], bf16, tag="shB")   # shared_batch^T (x2 if rowpack)
    shC = lat.tile([shp, rows], bf16, tag="shC")   # shared_cell^T
    zT1 = lat.tile([128, rows], bf16, tag="zT1")   # [h_base^T ; dec_b^T]
    zT2 = lat.tile([64, rows], bf16, tag="zT2")    # dec_c^T

    xpool = ctx.enter_context(tc.tile_pool(name="xT", bufs=xbufs))
    encps = ctx.enter_context(tc.tile_pool(name="encps", bufs=enc_bufs, space="PSUM"))
    headps = ctx.enter_context(tc.tile_pool(name="headps", bufs=hp_bufs, space="PSUM"))
    tpps = headps if tp_share else ctx.enter_context(
        tc.tile_pool(name="tpps", bufs=2, space="PSUM"))
    outps = ctx.enter_context(tc.tile_pool(name="outps", bufs=lp_bufs, space="PSUM"))
    tmpp = ctx.enter_context(tc.tile_pool(name="tmpp", bufs=tmp_bufs))
    small = ctx.enter_context(tc.tile_pool(name="small", bufs=small_bufs))
    opool = ctx.enter_context(tc.tile_pool(name="osb", bufs=osb_bufs))

    def select(ps_tiles, mask_t, moff, card, tag, out_ap):
        """Per-row expert selection: out[p, m] = sum_e mask[p, e] * ps[p, e*64+m].

        ps_tiles: list of ([128, w] psum AP, expert offset, n_experts).
        Writes the [128, 64] result to out_ap (bf16).
        """
        if ab == "nosel":
            nc.vector.tensor_copy(out_ap, ps_tiles[0][0][:, 0:64])
            return
        if fused_sel:
            # masked products land in [m, e] layout; one innermost-axis
            # reduce replaces the whole expert add-tree (out must be f32)
            tmp = tmpp.tile([128, 64 * card], bf16, tag=f"tmp{tag}")

            def _mk(base, dims):
                return bass.AP(base.tensor, base.offset, [base.ap[0]] + dims)

            for ps, e0, ne in ps_tiles:
                nc.vector.tensor_mul(
                    _mk(tmp[:, e0:], [[card, 64], [1, ne]]),
                    _mk(ps, [[1, 64], [64, ne]]),
                    _mk(mask_t[:, moff + e0:], [[0, 64], [1, ne]]),
                )
            nc.vector.reduce_sum(
                out_ap, _mk(tmp[:], [[card, 64], [1, card]]),
                axis=mybir.AxisListType.X)
            return
        tmp = tmpp.tile([128, card * 64], bf16, tag=f"tmp{tag}")
        if ab == "natsel":
            for ps, e0, ne in ps_tiles:
                for j in range(ne):
                    e = e0 + j
                    nc.vector.tensor_scalar_mul(
                        tmp[:, e * 64:(e + 1) * 64],
                        ps[:, j * 64:(j + 1) * 64],
                        mask_t[:, moff + e:moff + e + 1])
        else:
            for ps, e0, ne in ps_tiles:
                nc.vector.tensor_mul(
                    _ap3(tmp[:, e0 * 64:(e0 + ne) * 64], ne, 64),
                    _ap3(ps, ne, 64),
                    _bcast(mask_t[:, moff + e0:moff + e0 + ne], 64),
                )
        if dma_red:
            # expert reduction on the DMA engines: one SWDGE transfer with a
            # step-0 destination accumulates all expert blocks (per-partition
            # descriptor order makes the read-modify-write race-free)
            nc.gpsimd.memset(out_ap, 0.0)
            in3 = _ap3(tmp[:], card, 64)
            out3 = bass.AP(out_ap.tensor, out_ap.offset,
                           [out_ap.ap[0], [0, card], [1, 64]])
            nc.gpsimd.dma_start(out3, in3, accum_op=mybir.AluOpType.add)
            return
        # tree-reduce over experts
        eng = nc.gpsimd if adds_gpsimd else nc.vector

        def halve(src, n):
            h = n // 2
            dst = small.tile([128, h * 64], bf16, tag=f"acc{tag}{h}")
            eng.tensor_add(dst[:], src[:, :h * 64], src[:, h * 64:2 * h * 64])
            return dst, src[:, 2 * h * 64:] if n % 2 else None

        cur, n = tmp, card
        extras = []
        while n > 1:
            cur, rem = halve(cur, n)
            if rem is not None:
                extras.append(rem)
            n //= 2
        cur = cur[:]
        if extras:
            for ex in extras[:-1]:
                nxt = small.tile([128, 64], bf16, tag=f"hx{tag}")
                eng.tensor_add(nxt[:], cur, ex)
                cur = nxt[:]
            eng.tensor_add(out_ap, cur, extras[-1])
        else:
            eng.tensor_copy(out_ap, cur)

    def head_chunks(src2, b, wst_t, card):
        """All-experts matmuls; with rowpack, consecutive chunks go to
        alternating PE row-groups (partitions 0:64 / 64:128) and run
        concurrently."""
        res = []
        total = card * 64
        c0 = 0
        ci = 0
        while c0 < total:
            w = min(512, total - c0)
            ps = headps.tile([128, 512], f32, tag="hps")
            half = (ci % 2) * 64 if rowpack else 0
            lhsT = (src2[half:half + 64, b:b + 128] if b is not None
                    else src2[half:half + 64, :])
            nc.tensor.matmul(ps[:, :w], lhsT,
                             wst_t[half:half + 64, c0:c0 + w],
                             start=True, stop=True)
            res.append((ps[:, :w], c0 // 64, w // 64))
            c0 += w
            ci += 1
        return res

    def transpose_pair(src_t, tag):
        """[128, 128] sbuf pair -> [128, 128] psum via one PE transpose."""
        dt = f32 if fused_sel else bf16
        tp = tpps.tile([128, 128], dt, tag="hps" if tp_share else "tp")
        nc.tensor.transpose(tp[:], src_t[:],
                            ident32_t[:] if fused_sel else ident_t[:])
        return tp

    cp = mybir.ActivationFunctionType.Copy
    state = {}

    def encode_quarter(q):
        b0 = q * qw
        h1 = encps.tile([128, qw], f32, tag="h1")
        h2 = encps.tile([64, qw], f32, tag="h2")
        for k in range(NK):
            xt = xpool.tile([128, qw], bf16, tag="xt")
            nc.sync.dma_start(xt[:], x[k * 128:(k + 1) * 128, b0:b0 + qw])
            wt = wenc_ts[k // ws]
            ko = (k % ws) * 192
            for c0 in range(0, qw, 512):
                cw = min(512, qw - c0)
                nc.tensor.matmul(h1[:, c0:c0 + cw], wt[:, ko:ko + 128],
                                 xt[:, c0:c0 + cw],
                                 start=(k == 0), stop=(k == NK - 1))
                nc.tensor.matmul(h2[:, c0:c0 + cw], wt[:, ko + 128:ko + 192],
                                 xt[:, c0:c0 + cw],
                                 start=(k == 0), stop=(k == NK - 1))
        nc.scalar.activation(zT1[0:64, b0:b0 + qw], h1[0:64, :], cp)
        nc.scalar.activation(shB[0:64, b0:b0 + qw], h1[64:128, :], cp)
        nc.scalar.activation(shC[0:64, b0:b0 + qw], h2[0:64, :], cp)
        if rowpack:
            nc.scalar.activation(shB[64:128, b0:b0 + qw], h1[64:128, :], cp)
            nc.scalar.activation(shC[64:128, b0:b0 + qw], h2[0:64, :], cp)

    def phase1(t):
        """src heads: all-experts mm + select -> hfp [128, 128]."""
        b = t * 128
        hfp = small.tile([128, 128], f32 if fused_sel else bf16, tag="hfp")
        ps1 = head_chunks(shB, b, wstb_t, CARD_B)
        select(ps1, msbs_t, t * CARD_B, CARD_B, "b1", hfp[:, 0:64])
        ps1 = head_chunks(shC, b, wstc_t, CARD_C)
        select(ps1, mscs_t, t * CARD_C, CARD_C, "c1", hfp[:, 64:128])
        state[t] = hfp

    def phase2(t):
        """transpose hfp, tgt heads: mm + select -> dcp."""
        hfp = state.pop(t)
        hfT = transpose_pair(hfp, "s1")
        hp = 128 if rowpack else 64
        hfTb = small.tile([hp, 128], bf16, tag="hfTb")
        hfTc = small.tile([hp, 128], bf16, tag="hfTc")
        nc.scalar.activation(hfTb[0:64, :], hfT[0:64, :], cp)
        nc.scalar.activation(hfTc[0:64, :], hfT[64:128, :], cp)
        if rowpack:
            nc.scalar.activation(hfTb[64:128, :], hfT[0:64, :], cp)
            nc.scalar.activation(hfTc[64:128, :], hfT[64:128, :], cp)
        dcp = small.tile([128, 128], f32 if fused_sel else bf16, tag="dcp")
        ps2 = head_chunks(hfTb, None, wstb_t, CARD_B)
        select(ps2, msbt_t, t * CARD_B, CARD_B, "b2", dcp[:, 0:64])
        ps2 = head_chunks(hfTc, None, wstc_t, CARD_C)
        select(ps2, msct_t, t * CARD_C, CARD_C, "c2", dcp[:, 64:128])
        state[t] = dcp

    def phase3(t):
        """transpose dcp into zT, decoder matmuls, drain, store."""
        b = t * 128
        if ab == "nohead":
            nc.vector.tensor_copy(zT1[64:128, b:b + 128], shB[:, b:b + 128])
            nc.vector.tensor_copy(zT2[:, b:b + 128], shC[:, b:b + 128])
        else:
            dcp = state.pop(t)
            dcT = transpose_pair(dcp, "s2")
            nc.scalar.activation(zT1[64:128, b:b + 128], dcT[0:64, :], cp)
            nc.scalar.activation(zT2[:, b:b + 128], dcT[64:128, :], cp)
        osb = opool.tile([128, D], bf16, tag="osb")
        for n in range(NOUT):
            lp = outps.tile([128, OW], f32, tag="lp")
            nc.tensor.matmul(lp[:], zT1[:, b:b + 128],
                             wout1_t[:, n * OW:(n + 1) * OW],
                             start=True, stop=False)
            nc.tensor.matmul(lp[:], zT2[:, b:b + 128],
                             wout2_t[:, n * OW:(n + 1) * OW],
                             start=False, stop=True)
            if n % NOUT < lp_dve:
                nc.vector.tensor_copy(osb[:, n * OW:(n + 1) * OW], lp[:])
            else:
                nc.scalar.activation(osb[:, n * OW:(n + 1) * OW], lp[:], cp)
        nc.scalar.dma_start(y[b:b + 128, :], osb[:])

    def tiles_of(q):
        return range(q * (qw // 128), (q + 1) * (qw // 128))

    for q in range(nq):
        encode_quarter(q)
        # software pipeline (depth 2) so PE never stalls on a single
        # tile's select chain: stage-1 of tile t issues ahead of
        # stage-2 of t-1 and the decoder of t-2.
        if ab == "nohead":
            for t in tiles_of(q):
                phase3(t)
        else:
            for t in tiles_of(q):
                phase1(t)
                if t - lag1 >= 0:
                    phase2(t - lag1)
                if t - lag2 >= 0:
                    phase3(t - lag2)
    if ab != "nohead":
        for t in range(nt - lag1, nt):
            phase2(t)
        for t in range(nt - lag2, nt):
            phase3(t)


def _declare(nc, rows):
    def di(name, shape, dt):
        return nc.dram_tensor(name, shape, dt, kind="ExternalInput").ap()

    x = di("x", [DP, rows], bf16)   # expr^T (host pre-transposed)
    wenc = di("wenc", [128, NK * 192], bf16)
    wout1 = di("wout1", [128, D], bf16)
    wout2 = di("wout2", [64, D], bf16)
    wstb = di("wstb", [128, CARD_B * 64], bf16)
    wstc = di("wstc", [128, CARD_C * 64], bf16)
    nt = rows // 128
    msbs = di("msbs", [128, nt * CARD_B], f32)
    msbt = di("msbt", [128, nt * CARD_B], f32)
    mscs = di("mscs", [128, nt * CARD_C], f32)
    msct = di("msct", [128, nt * CARD_C], f32)
    ident = di("ident", [128, 128], bf16)
    ident32 = di("ident32", [128, 128], f32)
    y = nc.dram_tensor("y", [rows, D], bf16, kind="ExternalOutput").ap()
    return [x, wenc, wout1, wout2, wstb, wstc, msbs, msbt, mscs, msct,
            ident, ident32, y]


_PROGRAMS = {}


def build_program(rows=R, ab="full", **kw):
    key = (rows, ab, tuple(sorted(kw.items())))
    if key in _PROGRAMS:
        return _PROGRAMS[key]
    nc = bacc.Bacc("TRN2", target_bir_lowering=False, debug=False,
                   num_devices=N_CORES if rows == R else 1)
    io = _declare(nc, rows)
    with tile.TileContext(nc) as tc:
        _kernel(tc, rows, io, ab=ab, **kw)
    nc.compile()
    _PROGRAMS[key] = nc
    return nc


def prep_weights(W_base, W_enc_batch, W_dec_batch, W_heads_batch,
                 W_enc_cell, W_dec_cell, W_heads_cell):
    stackT = np.zeros((DP, 192), np.float32)
    stackT[:D, 0:64] = W_base.T
    stackT[:D, 64:128] = W_enc_batch.T
    stackT[:D, 128:192] = W_enc_cell.T
    wenc = np.ascontiguousarray(
        stackT.reshape(NK, 128, 192).transpose(1, 0, 2).reshape(128, NK * 192)
    ).astype(BF16)
    wout = np.concatenate(
        [W_base, DEC_SCALE * W_dec_batch.T, DEC_SCALE * W_dec_cell.T], axis=0
    ).astype(BF16)
    wstb = np.ascontiguousarray(
        W_heads_batch.transpose(2, 0, 1).reshape(64, CARD_B * 64)).astype(BF16)
    wstb = np.vstack([wstb, wstb])
    wstc = np.ascontiguousarray(
        W_heads_cell.transpose(2, 0, 1).reshape(64, CARD_C * 64)).astype(BF16)
    wstc = np.vstack([wstc, wstc])
    ident = np.eye(128, dtype=BF16)
    return {
        "wenc": wenc, "wout1": np.ascontiguousarray(wout[0:128]),
        "wout2": np.ascontiguousarray(wout[128:192]),
        "wstb": wstb, "wstc": wstc, "ident": ident,
        "ident32": np.eye(128, dtype=np.float32),
    }


def prep_mask(idx, card):
    """[rows] int -> [128, (rows/128)*card] f32 one-hot in SBUF layout."""
    nt = idx.shape[0] // 128
    oh = (idx.reshape(nt, 128)[:, :, None] == np.arange(card)).astype(np.float32)
    return np.ascontiguousarray(oh.transpose(1, 0, 2).reshape(128, nt * card))


def prep_x(expr_rows):
    """[rows, D] f32 -> padded transposed [DP, rows] bf16."""
    xp = np.zeros((DP, expr_rows.shape[0]), BF16)
    xp[:D, :] = expr_rows.astype(BF16).T
    return xp


def kernel(expr, src_batch, tgt_batch, src_cell, tgt_cell,
           W_base, W_enc_batch, W_dec_batch, W_heads_batch,
           W_enc_cell, W_dec_cell, W_heads_cell):
    import os
    # The NTFF trace path needs antenv.axon_hooks, absent in this
    # container; a stray BASS_TRACE=1 would crash the run otherwise.
    os.environ.setdefault("BASS_NEVER_TRACE", "1")
    expr = np.asarray(expr)
    src_batch, tgt_batch = np.asarray(src_batch), np.asarray(tgt_batch)
    src_cell, tgt_cell = np.asarray(src_cell), np.asarray(tgt_cell)
    nc = build_program(R)
    wmap = prep_weights(W_base, W_enc_batch, W_dec_batch, W_heads_batch,
                        W_enc_cell, W_dec_cell, W_heads_cell)
    in_maps = []
    for c in range(N_CORES):
        sl = slice(c * R, (c + 1) * R)
        in_maps.append({
            "x": prep_x(expr[sl]),
            "msbs": prep_mask(src_batch[sl], CARD_B),
            "msbt": prep_mask(tgt_batch[sl], CARD_B),
            "mscs": prep_mask(src_cell[sl], CARD_C),
            "msct": prep_mask(tgt_cell[sl], CARD_C),
            **wmap,
        })
    res = run_bass_kernel_spmd(nc, in_maps, core_ids=list(range(N_CORES)))
    global LAST_RESULT
    LAST_RESULT = res
    out = np.concatenate([res.results[c]["y"] for c in range(N_CORES)], axis=0)
    return np.asarray(out, dtype=np.float32)


LAST_RESULT = None
